# revision 8
# baseline (speedup 1.0000x reference)
"""Trainium2 Bass kernel for nn_BipropLinear (topk column-masked sign-binarized linear).

Full inputs -> full outputs. Internally sharded over 8 NeuronCores on a 2x4 grid:
  - sample rows (batch*seq = 8192) split 2 ways  (im = 0, 1)
  - weight/score rows (d_out = 4096) split 4 ways (jn = 0..3)
Each core computes its [4096 x 1024] block of the output einsum, plus (redundantly
across im) the associate_memory / mask outputs for its jn row-shard.

Device-side computation per core:
  1. col-sum of score shard and abs-col-sum of weight shard (DVE accumulate +
     fp32 ones-matmul partition reduce), AllReduce across the 8 cores.
  2. exact rank selection of the n_drop=819 smallest column means via two
     gpsimd kth_largest calls (819 > the 510 heap cap, so two rounds).
  3. mask, scaling factor (sum(|w|*mask)/sum(mask)), sign(weight),
     associate_memory = sign(w) * ((mask - score) + score)  [bitwise-matching
     the reference's straight-through perturbation].
  4. out = scaling * sample @ (sign(w)*mask)^T computed as a split-precision
     matmul: sample = hi(fp16) + lo(bf16), sign matrix exact in bf16, fp32 PSUM
     accumulation -> ~3e-7 relative error vs the fp32 reference.
"""

import numpy as np

import concourse.bass as bass
import concourse.bass_isa as bass_isa
import concourse.mybir as mybir
import concourse.tile as tile
from concourse import bacc
from concourse.bass_utils import run_bass_kernel_spmd
from concourse.masks import make_identity

F32 = mybir.dt.float32
BF16 = mybir.dt.bfloat16
F16 = mybir.dt.float16
ALU = mybir.AluOpType
ACTF = mybir.ActivationFunctionType

B, S, D_IN, D_OUT = 4, 2048, 4096, 4096
GM, GN = 2, 4
M = B * S             # 8192
M_SH = M // GM        # 4096 sample rows per core
O_SH = D_OUT // GN    # 1024 weight/score rows per core
N_DROP = 819          # d_in - ceil((1-0.2)*d_in)
KT = D_IN // 128      # 32 k-tiles
MT = M_SH // 128      # 32 m-tiles
OTS = O_SH // 128     # 8 o-subtiles per shard

# two-round rank selection: drop set = N_DROP largest of x = -col_mean.
# round 1 removes the top (R1_KADJ+1) = 510 via a lerped threshold strictly
# inside (desc[509], desc[510]); round 2 finds desc[818..819] of the rest.
R1_KADJ = 509
R2_KADJ = N_DROP - 1 - (R1_KADJ + 1)   # 308
N_R2 = D_IN - (R1_KADJ + 1)            # 3586 valid values in round 2

_NC = None


def _build():
    nc = bacc.Bacc("TRN2", target_bir_lowering=False, debug=False,
                   num_devices=8)

    sample = nc.dram_tensor("sample", [M_SH, D_IN], F32, kind="ExternalInput")
    weight = nc.dram_tensor("weight", [O_SH, D_IN], F32, kind="ExternalInput")
    score = nc.dram_tensor("score", [O_SH, D_IN], F32, kind="ExternalInput")

    out_sh = nc.dram_tensor("out_shard", [M_SH, O_SH], F32, kind="ExternalOutput")
    assoc_sh = nc.dram_tensor("assoc_shard", [O_SH, D_IN], F32, kind="ExternalOutput")
    maskrow = nc.dram_tensor("mask_row", [1, D_IN], F32, kind="ExternalOutput")
    dbg = nc.dram_tensor("dbg", [1, 8], F32, kind="ExternalOutput")

    with tile.TileContext(nc) as tc:
        with (
            tc.tile_pool(name="small", bufs=1) as small,
            tc.tile_pool(name="dram", bufs=1, space="DRAM") as dram,
        ):
            ident = small.tile([128, 128], F32, tag="ident")
            make_identity(nc, ident[:])
            ones_neg = small.tile([128, 1], F32, tag="ones_neg")
            nc.vector.memset(ones_neg[:], -1.0 / (GM * D_OUT))
            ones_half = small.tile([128, 1], F32, tag="ones_half")
            nc.vector.memset(ones_half[:], 1.0 / GM)
            ones_row = small.tile([1, 128], BF16, tag="ones_row")
            nc.vector.memset(ones_row[:], 1.0)

            stats_in = dram.tile([2, D_IN], F32)
            stats_out = dram.tile([2, D_IN], F32)
            sg_scr = dram.tile([O_SH, D_IN], BF16)

            x_t = small.tile([128, KT], F32, tag="x_t")
            absw_t = small.tile([128, KT], F32, tag="absw_t")
            mask_t = small.tile([128, KT], F32, tag="mask_t")
            s_b = small.tile([128, 1], F32, tag="s_b")
            mask_row_sb = small.tile([1, D_IN], BF16, tag="mask_row_sb")
            mask_bc = small.tile([128, D_IN], BF16, tag="mask_bc")

            # ---------------- phase 1: stats + sign + selection --------
            with (
                tc.tile_pool(name="pstat", bufs=2) as pstat,
                tc.tile_pool(name="pacc", bufs=1) as pacc,
                tc.tile_pool(name="ps_stat", bufs=2, space="PSUM") as ps_stat,
            ):
                acc_s = pacc.tile([128, D_IN], F32, tag="acc_s")
                acc_w = pacc.tile([128, D_IN], F32, tag="acc_w")
                nc.vector.memset(acc_s[:], 0.0)
                nc.vector.memset(acc_w[:], 0.0)
                for ot in range(OTS):
                    st = pstat.tile([128, D_IN], F32, tag="st")
                    wt = pstat.tile([128, D_IN], F32, tag="wt")
                    nc.sync.dma_start(st[:], score[ot * 128:(ot + 1) * 128, :])
                    nc.sync.dma_start(wt[:], weight[ot * 128:(ot + 1) * 128, :])
                    # sign(w) as bf16 (exact +-1/0), to DRAM scratch
                    sgn_bf = pstat.tile([128, D_IN], BF16, tag="sgn_bf")
                    nc.scalar.sign(sgn_bf[:], wt[:])
                    nc.sync.dma_start(sg_scr[ot * 128:(ot + 1) * 128, :], sgn_bf[:])
                    # accumulate column stats across o-subtiles
                    nc.vector.tensor_tensor(
                        out=acc_s[:], in0=acc_s[:], in1=st[:], op=ALU.add)
                    aw = pstat.tile([128, D_IN], F32, tag="aw")
                    nc.scalar.activation(aw[:], wt[:], ACTF.Abs)
                    nc.vector.tensor_tensor(
                        out=acc_w[:], in0=acc_w[:], in1=aw[:], op=ALU.add)

                # partition-reduce via fp32 ones-matmuls, staged to DRAM:
                # row 0: -colsum(score)/(GM*D_OUT); row 1: colsum(|w|)/GM
                for c in range(8):
                    sl = slice(c * 512, (c + 1) * 512)
                    p1 = ps_stat.tile([1, 512], F32, tag="ps_cm")
                    nc.tensor.matmul(p1[:], ones_neg[:], acc_s[:, sl],
                                     start=True, stop=True)
                    stg1 = pstat.tile([1, 512], F32, tag="stage")
                    nc.scalar.copy(stg1[:], p1[:])
                    nc.sync.dma_start(stats_in[0:1, sl], stg1[:])
                    p2 = ps_stat.tile([1, 512], F32, tag="ps_aw")
                    nc.tensor.matmul(p2[:], ones_half[:], acc_w[:, sl],
                                     start=True, stop=True)
                    stg2 = pstat.tile([1, 512], F32, tag="stage")
                    nc.scalar.copy(stg2[:], p2[:])
                    nc.sync.dma_start(stats_in[1:2, sl], stg2[:])

                nc.gpsimd.collective_compute(
                    "AllReduce", ALU.add,
                    replica_groups=[list(range(8))],
                    ins=[stats_in.opt()],
                    outs=[stats_out.opt()],
                )

                # x = -col_mean in [128, 32] layout with i = f*128 + p
                nc.sync.dma_start(
                    x_t[:],
                    stats_out[0:1, :].rearrange("o (f p) -> (o p) f", p=128))
                nc.sync.dma_start(
                    absw_t[:],
                    stats_out[1:2, :].rearrange("o (f p) -> (o p) f", p=128))
                x_row = pacc.tile([1, D_IN], F32, tag="x_row")
                nc.sync.dma_start(x_row[:], stats_out[0:1, :])

                # ---- rank selection (two gpsimd kth_largest rounds)
                kv1 = small.tile([128, 2], F32, tag="kv1")
                nc.gpsimd.kth_largest(
                    kv1[:], x_t[:], n_per_lane=KT, k=R1_KADJ,
                    quantile=1.0 - (R1_KADJ + 0.5) / (D_IN - 1))
                t1b = small.tile([128, 1], F32, tag="t1b")
                nc.gpsimd.partition_broadcast(t1b[:], kv1[:1, 0:1])
                gem = small.tile([128, KT], mybir.dt.uint32, tag="gem")
                nc.vector.tensor_scalar(gem[:], x_t[:], t1b[:], None,
                                        op0=ALU.is_ge)
                negbig = small.tile([128, KT], F32, tag="negbig")
                nc.vector.memset(negbig[:], -1.0e30)
                x2 = small.tile([128, KT], F32, tag="x2")
                nc.vector.tensor_copy(x2[:], x_t[:])
                nc.vector.copy_predicated(x2[:], gem[:], negbig[:])
                kv2 = small.tile([128, 2], F32, tag="kv2")
                nc.gpsimd.kth_largest(
                    kv2[:], x2[:], n_per_lane=KT, k=R2_KADJ + 2,
                    quantile=1.0 - (R2_KADJ + 0.5) / (N_R2 - 1))
                tb = small.tile([128, 1], F32, tag="tb")
                nc.gpsimd.partition_broadcast(tb[:], kv2[:1, 0:1])

                # mask (keep) = x < T ; cnt = #kept per partition
                cnt_p = small.tile([128, 1], F32, tag="cnt_p")
                nc.vector.tensor_scalar(mask_t[:], x_t[:], tb[:], 0.0,
                                        op0=ALU.is_lt, op1=ALU.add,
                                        accum_out=cnt_p[:])
                num_t = small.tile([128, KT], F32, tag="num_t")
                num_p = small.tile([128, 1], F32, tag="num_p")
                nc.vector.scalar_tensor_tensor(
                    out=num_t[:], in0=mask_t[:], scalar=0.0, in1=absw_t[:],
                    op0=ALU.bypass, op1=ALU.mult, accum_out=num_p[:])
                cnt = small.tile([128, 1], F32, tag="cnt")
                num = small.tile([128, 1], F32, tag="num")
                nc.gpsimd.partition_all_reduce(
                    cnt[:], cnt_p[:], channels=128,
                    reduce_op=bass_isa.ReduceOp.add)
                nc.gpsimd.partition_all_reduce(
                    num[:], num_p[:], channels=128,
                    reduce_op=bass_isa.ReduceOp.add)
                # scaling = num / (cnt * D_OUT): reciprocal + one Newton step
                d = small.tile([128, 1], F32, tag="d")
                nc.vector.tensor_scalar_mul(d[:], cnt[:], float(D_OUT))
                r0 = small.tile([128, 1], F32, tag="r0")
                nc.vector.reciprocal(r0[:], d[:])
                e = small.tile([128, 1], F32, tag="e")
                nc.vector.tensor_tensor(out=e[:], in0=d[:], in1=r0[:],
                                        op=ALU.mult)
                f2 = small.tile([128, 1], F32, tag="f2")
                nc.vector.tensor_scalar(f2[:], e[:], -1.0, 2.0, op0=ALU.mult,
                                        op1=ALU.add)
                rr = small.tile([128, 1], F32, tag="rr")
                nc.vector.tensor_tensor(out=rr[:], in0=r0[:], in1=f2[:],
                                        op=ALU.mult)
                nc.vector.tensor_tensor(out=s_b[:], in0=num[:], in1=rr[:],
                                        op=ALU.mult)

                # mask in row layout (bf16; 0/1 exact)
                nc.vector.tensor_scalar(mask_row_sb[:], x_row[:],
                                        kv2[:1, 0:1], None, op0=ALU.is_lt)
                # debug scalars
                dbg_sb = small.tile([1, 8], F32, tag="dbg_sb")
                nc.vector.tensor_copy(dbg_sb[:, 0:2], kv2[:1, :])
                nc.vector.tensor_copy(dbg_sb[:, 2:3], cnt[:1, :])
                nc.vector.tensor_copy(dbg_sb[:, 3:4], num[:1, :])
                nc.vector.tensor_copy(dbg_sb[:, 4:5], s_b[:1, :])
                nc.vector.tensor_copy(dbg_sb[:, 5:7], kv1[:1, :])
                nc.vector.tensor_copy(dbg_sb[:, 7:8], e[:1, :])
                nc.sync.dma_start(dbg[:], dbg_sb[:])

            with tc.tile_pool(name="ps_bc", bufs=2, space="PSUM") as ps_bc:
                for c in range(8):
                    sl = slice(c * 512, (c + 1) * 512)
                    pb = ps_bc.tile([128, 512], F32, tag="pb")
                    nc.tensor.matmul(pb[:], ones_row[:], mask_row_sb[:, sl],
                                     start=True, stop=True)
                    nc.scalar.copy(mask_bc[:, sl], pb[:])

            # ---------------- phase 3: main + assoc --------------------
            with (
                tc.tile_pool(name="psgn", bufs=1) as psgn,
                tc.tile_pool(name="pmain", bufs=2) as pmain,
                tc.tile_pool(name="pchunk", bufs=3) as pchunk,
                tc.tile_pool(name="pmm", bufs=2, space="PSUM") as pmm,
            ):
                # masked sign^T, bf16, resident: [128(k part), KT, O_SH]
                sgnT = psgn.tile([128, KT, O_SH], BF16, tag="sgnT")
                for kt in range(KT):
                    nc.sync.dma_start(
                        sgnT[:, kt, :],
                        sg_scr[:, kt * 128:(kt + 1) * 128],
                        transpose=True)
                    nc.vector.tensor_scalar_mul(
                        sgnT[:, kt, :], sgnT[:, kt, :], mask_t[:, kt:kt + 1])

                def assoc_piece(ot, h):
                    rows = slice(ot * 128, (ot + 1) * 128)
                    cols = slice(h * 2048, (h + 1) * 2048)
                    st2 = pmain.tile([128, 2048], F32, tag="st2")
                    nc.sync.dma_start(st2[:], score[rows, cols])
                    sgn_n = pmain.tile([128, 2048], BF16, tag="sgn_n")
                    nc.sync.dma_start(sgn_n[:], sg_scr[rows, cols])
                    pm = pmain.tile([128, 2048], F32, tag="pm")
                    nc.vector.tensor_tensor(out=pm[:], in0=mask_bc[:, cols],
                                            in1=st2[:], op=ALU.subtract)
                    nc.vector.tensor_tensor(out=pm[:], in0=pm[:], in1=st2[:],
                                            op=ALU.add)
                    if ot == 0:
                        nc.sync.dma_start(maskrow[0:1, cols], pm[0:1, :])
                    nc.vector.tensor_tensor(out=pm[:], in0=sgn_n[:], in1=pm[:],
                                            op=ALU.mult)
                    nc.sync.dma_start(assoc_sh[rows, cols], pm[:])

                assoc_jobs = [(ot, h) for ot in range(OTS) for h in range(2)]

                for mt in range(MT):
                    hiT = pmain.tile([128, D_IN], F16, tag="hiT")
                    loT = pmain.tile([128, D_IN], BF16, tag="loT")
                    mrows = slice(mt * 128, (mt + 1) * 128)
                    for q in range(8):
                        sc = pchunk.tile([128, 512], F32, tag="sc")
                        nc.sync.dma_start(
                            sc[:], sample[mrows, q * 512:(q + 1) * 512])
                        pt = pmm.tile([128, 512], F32, tag="ps_t")
                        for j in range(4):
                            nc.tensor.matmul(
                                pt[:, j * 128:(j + 1) * 128],
                                sc[:, j * 128:(j + 1) * 128],
                                ident[:],
                                is_transpose=True,
                                start=(j == 0), stop=(j == 3))
                        qsl = slice(q * 512, (q + 1) * 512)
                        nc.scalar.copy(hiT[:, qsl], pt[:])
                        nc.vector.tensor_tensor(
                            out=loT[:, qsl], in0=pt[:], in1=hiT[:, qsl],
                            op=ALU.subtract)
                    for o2 in range(2):
                        po = pmm.tile([128, 512], F32, tag="ps_o")
                        osl = slice(o2 * 512, (o2 + 1) * 512)
                        for si, split in enumerate((hiT, loT)):
                            for kt in range(KT):
                                nc.tensor.matmul(
                                    po[:],
                                    split[:, kt * 128:(kt + 1) * 128],
                                    sgnT[:, kt, osl],
                                    start=(si == 0 and kt == 0),
                                    stop=(si == 1 and kt == KT - 1))
                        ob = pmain.tile([128, 512], F32, tag="ob")
                        nc.scalar.activation(ob[:], po[:], ACTF.Copy,
                                             scale=s_b[:])
                        nc.sync.dma_start(out_sh[mrows, osl], ob[:])
                    # interleave one assoc piece every other m-tile
                    if mt % 2 == 0 and assoc_jobs:
                        assoc_piece(*assoc_jobs.pop(0))
                while assoc_jobs:
                    assoc_piece(*assoc_jobs.pop(0))

    nc.compile()
    return nc


def _get_nc():
    global _NC
    if _NC is None:
        _NC = _build()
    return _NC


LAST_RUN_INFO = {}


def kernel(sample, weight, score):
    import time
    nc = _get_nc()
    sample2d = np.ascontiguousarray(
        np.asarray(sample, dtype=np.float32).reshape(M, D_IN))
    weight = np.asarray(weight, dtype=np.float32)
    score = np.asarray(score, dtype=np.float32)

    in_maps = []
    for c in range(8):
        im, jn = divmod(c, GN)
        in_maps.append({
            "sample": sample2d[im * M_SH:(im + 1) * M_SH],
            "weight": weight[jn * O_SH:(jn + 1) * O_SH],
            "score": score[jn * O_SH:(jn + 1) * O_SH],
        })

    t0 = time.time()
    res = run_bass_kernel_spmd(nc, in_maps, core_ids=list(range(8)))
    LAST_RUN_INFO["spmd_wall_s"] = time.time() - t0
    r = res.results

    out = np.empty((M, D_OUT), np.float32)
    for c in range(8):
        im, jn = divmod(c, GN)
        out[im * M_SH:(im + 1) * M_SH,
            jn * O_SH:(jn + 1) * O_SH] = r[c]["out_shard"]
    assoc = np.concatenate([r[jn]["assoc_shard"] for jn in range(GN)], axis=0)
    mask_row = r[0]["mask_row"][0]
    LAST_RUN_INFO["dbg"] = r[0]["dbg"]
    return (out.reshape(B, S, D_OUT), assoc, mask_row)
